# revision 4
# baseline (speedup 1.0000x reference)
"""Trainium2 Bass kernel for nn_Attention_51307679318359.

Multi-head attention (B=2, S=2048, D=2048, H=16, HD=128) with RoPE and an
additive mask, sharded over 8 NeuronCores as (batch x head-group): each core
computes 1 batch and 4 heads (512 channels), producing a partial output that
the host sums over head-groups.

Per-core dataflow (all activations kept transposed, channels on partitions):
  QT = wqT' @ xT, KT = wkT' @ xT  (rotate-half permuted weights), RoPE applied
  on the PSUM output via DVE; V = xT' @ wvT; all spilled to DRAM as f32r.
  Per head: scoresT(sk,sq) = KT_h^T-slices @ QT_h (fp32r, one matmul per
  128x512 block), additive mask patterns on the diagonal blocks, exp on ACT,
  AV + denominator accumulated in PSUM (fp32r matmuls, ones-vector reduction),
  normalize with DVE using a K=1 fp32 broadcast matmul. Out-proj contracts the
  4 head tiles against woT.
Fully-masked score blocks (causal upper triangle) are skipped on host evidence:
the mask input is classified per 128x512 block into skip / plain / pattern.
"""

import math

import numpy as np

import concourse.bass as bass
import concourse.mybir as mybir
import concourse.tile as tile
from concourse import bacc
from concourse import bass_utils

F32 = mybir.dt.float32
F32R = mybir.dt.float32r
ADD = mybir.AluOpType.add
MULT = mybir.AluOpType.mult

B, S, D = 2, 2048, 2048
H, HD = 16, 128
NCORES = 8
GROUPS = NCORES // B          # 4 head-groups
HPG = H // GROUPS             # 4 heads per group
C = HPG * HD                  # 512 per-core channels
P = 128
CH_A = 256                    # phase-A s-chunk width
SQ = 512                      # phase-B sq-chunk width
SCALE = 1.0 / math.sqrt(HD)
NEG_THRESH = -1e8             # "masked out" threshold

_PROGRAM_CACHE = {}


def _classify_mask(mask):
    """Classify transposed-mask blocks (sk-tile i x sq-chunk j) and dedupe the
    mixed patterns. Returns (classes, patterns) where classes[(j, i)] is
    'skip' | 'plain' | int pattern index, patterns is (nblk, 128, SQ) f32
    already divided by SCALE (so exp(SCALE*(qk + pat)) == exp(SCALE*qk + mask)).
    """
    maskT = np.ascontiguousarray(mask.T)
    n_j = mask.shape[0] // SQ
    n_i = mask.shape[0] // P
    classes = {}
    patterns = []
    pat_idx = {}
    for j in range(n_j):
        for i in range(n_i):
            blk = maskT[i * P:(i + 1) * P, j * SQ:(j + 1) * SQ]
            if np.all(blk == 0.0):
                classes[(j, i)] = 'plain'
            elif np.all(blk <= NEG_THRESH):
                classes[(j, i)] = 'skip'
            else:
                key = blk.tobytes()
                if key not in pat_idx:
                    pat_idx[key] = len(patterns)
                    patterns.append(blk / np.float32(SCALE))
                classes[(j, i)] = pat_idx[key]
    # every sq position must keep at least one live sk tile, else softmax
    # denominators vanish; fall back to no skipping in that degenerate case
    for j in range(n_j):
        if all(classes[(j, i)] == 'skip' for i in range(n_i)):
            for jj in range(n_j):
                for i in range(n_i):
                    if classes[(jj, i)] == 'skip':
                        blk = maskT[i * P:(i + 1) * P, jj * SQ:(jj + 1) * SQ]
                        key = blk.tobytes()
                        if key not in pat_idx:
                            pat_idx[key] = len(patterns)
                            patterns.append(blk / np.float32(SCALE))
                        classes[(jj, i)] = pat_idx[key]
            break
    pats = np.stack(patterns, 0).astype(np.float32) if patterns else \
        np.zeros((1, P, SQ), np.float32)
    return classes, pats


def _build(classes, nblk, s=S, d=D):
    """Build + compile the per-core SPMD program."""
    nkt = d // P
    n_j = s // SQ
    n_i = s // P
    n_ja = s // CH_A

    nc = bacc.Bacc("TRN2", target_bir_lowering=False, debug=False)
    xT = nc.dram_tensor("xT", (d, s), F32, kind="ExternalInput")
    wqT = nc.dram_tensor("wqT", (d, C), F32, kind="ExternalInput")
    wkT = nc.dram_tensor("wkT", (d, C), F32, kind="ExternalInput")
    wvT = nc.dram_tensor("wvT", (d, C), F32, kind="ExternalInput")
    woT = nc.dram_tensor("woT", (C, d), F32, kind="ExternalInput")
    cosP = nc.dram_tensor("cosP", (HD, s), F32, kind="ExternalInput")
    sinSw = nc.dram_tensor("sinSw", (HD, s), F32, kind="ExternalInput")
    mblk = nc.dram_tensor("mblk", (nblk, P, SQ), F32, kind="ExternalInput")
    onesd = nc.dram_tensor("onesd", (P, 1), F32, kind="ExternalInput")
    out = nc.dram_tensor("out", (s, d), F32, kind="ExternalOutput")

    with tile.TileContext(nc) as tc:
        with tc.tile_pool(name="dram", bufs=1, space="DRAM") as dram, \
             tc.tile_pool(name="const", bufs=1) as const, \
             tc.tile_pool(name="attn", bufs=1) as attnp:
            qt_d = dram.tile([C, s], F32R)
            kt_d = dram.tile([C, s], F32R)
            v_d = dram.tile([s, C], F32R)

            ones_r = const.tile([P, 1], F32R)
            nc.sync.dma_start(ones_r[:], onesd[:].bitcast(F32R))
            ones_f = const.tile([1, P], F32)
            nc.sync.dma_start(ones_f[:], onesd[:].rearrange("a b -> b a"))
            mblk_t = const.tile([P, nblk, SQ], F32)
            nc.sync.dma_start(mblk_t[:], mblk[:].rearrange("n p q -> p n q"))

            attn_t = attnp.tile([P, HPG, s], F32R)

            # ---------------- Phase A: QKV projections + RoPE ----------------
            with tc.tile_pool(name="wres", bufs=1) as wres, \
                 tc.tile_pool(name="xc", bufs=2) as xcp, \
                 tc.tile_pool(name="trig", bufs=1) as trig, \
                 tc.tile_pool(name="ptmp", bufs=3) as ptmp, \
                 tc.tile_pool(name="stg", bufs=4) as stg, \
                 tc.tile_pool(name="psA", bufs=2, space="PSUM") as psA:
                cos_t = trig.tile([P, s], F32)
                nc.sync.dma_start(cos_t[:], cosP[:])
                sin_t = trig.tile([P, s], F32)
                nc.sync.dma_start(sin_t[:], sinSw[:])
                wq_t = wres.tile([P, nkt, C], F32R)
                nc.sync.dma_start(
                    wq_t[:], wqT[:].rearrange("(ko p) c -> p ko c", p=P).bitcast(F32R))
                wk_t = wres.tile([P, nkt, C], F32R)
                nc.sync.dma_start(
                    wk_t[:], wkT[:].rearrange("(ko p) c -> p ko c", p=P).bitcast(F32R))
                wv_t = wres.tile([P, nkt, C], F32R)
                nc.sync.dma_start(
                    wv_t[:], wvT[:].rearrange("(ko p) c -> p ko c", p=P).bitcast(F32R))

                for j in range(n_ja):
                    sl = slice(j * CH_A, (j + 1) * CH_A)
                    xc = xcp.tile([P, nkt, CH_A], F32R, tag="xc")
                    nc.sync.dma_start(
                        xc[:],
                        xT[:].rearrange("(ko p) t -> p ko t", p=P)[:, :, sl].bitcast(F32R))
                    for (wt, dst) in ((wq_t, qt_d), (wk_t, kt_d)):
                        for ct in range(HPG):
                            ps = psA.tile([P, CH_A], F32, tag="ps_qk")
                            for k in range(nkt):
                                nc.tensor.matmul(
                                    ps[:], wt[:, k, ct * P:(ct + 1) * P],
                                    xc[:, k, :],
                                    start=(k == 0), stop=(k == nkt - 1))
                            # RoPE (rotate-half layout):
                            #   out_top = x0*cos - x1*sin ; out_bot = x1*cos + x0*sin
                            t1 = ptmp.tile([P, CH_A], F32, tag="t1")
                            nc.vector.tensor_tensor(t1[:], ps[:], cos_t[:, sl], MULT)
                            t2 = ptmp.tile([P, CH_A], F32, tag="t2")
                            nc.vector.tensor_tensor(
                                t2[0:64, :], ps[64:128, :], sin_t[64:128, sl], MULT)
                            nc.vector.tensor_tensor(
                                t2[64:128, :], ps[0:64, :], sin_t[0:64, sl], MULT)
                            ro = stg.tile([P, CH_A], F32R, tag="ro")
                            nc.vector.tensor_tensor(ro[:], t1[:], t2[:], ADD)
                            nc.sync.dma_start(dst[ct * P:(ct + 1) * P, sl], ro[:])
                    for st2 in range(CH_A // P):
                        st = (j * CH_A) // P + st2
                        psv = psA.tile([P, C], F32, tag="ps_v")
                        for k in range(nkt):
                            nc.tensor.matmul(
                                psv[:], xc[:, k, st2 * P:(st2 + 1) * P],
                                wv_t[:, k, :],
                                start=(k == 0), stop=(k == nkt - 1))
                        vo = stg.tile([P, C], F32R, tag="vo")
                        nc.vector.tensor_copy(vo[:], psv[:])
                        nc.sync.dma_start(v_d[st * P:(st + 1) * P, :], vo[:])

            # ---------------- Phase B: attention per head ----------------
            with tc.tile_pool(name="hq", bufs=2) as hqp, \
                 tc.tile_pool(name="pr", bufs=4) as prp, \
                 tc.tile_pool(name="sm", bufs=2) as smp, \
                 tc.tile_pool(name="psB", bufs=2, space="PSUM") as psB, \
                 tc.tile_pool(name="psB1", bufs=1, space="PSUM") as psB1:
                for h in range(HPG):
                    qh = hqp.tile([P, s], F32R, tag="qh")
                    nc.sync.dma_start(qh[:], qt_d[h * P:(h + 1) * P, :])
                    kh = hqp.tile([P, s], F32R, tag="kh")
                    nc.sync.dma_start(kh[:], kt_d[h * P:(h + 1) * P, :])
                    vh = hqp.tile([P, nkt_s := s // P, HD], F32R, tag="vh")
                    nc.sync.dma_start(
                        vh[:],
                        v_d.rearrange("(ko p) c -> p ko c", p=P)[:, :, h * HD:(h + 1) * HD])
                    for jq in range(n_j):
                        live = [i for i in range(n_i) if classes[(jq, i)] != 'skip']
                        qsl = slice(jq * SQ, (jq + 1) * SQ)
                        at_ps = psB.tile([P, SQ], F32, tag="at")
                        dn_ps = psB.tile([1, SQ], F32, tag="dn")
                        for n, i in enumerate(live):
                            sc = psB.tile([P, SQ], F32, tag="sc")
                            nc.tensor.matmul(
                                sc[:], kh[:, i * P:(i + 1) * P], qh[:, qsl],
                                start=True, stop=True)
                            cls = classes[(jq, i)]
                            if isinstance(cls, int):
                                nc.vector.tensor_tensor(
                                    sc[:], sc[:], mblk_t[:, cls, :], ADD)
                            pr = prp.tile([P, SQ], F32R, tag="pr")
                            nc.scalar.activation(
                                pr[:], sc[:], mybir.ActivationFunctionType.Exp,
                                scale=SCALE)
                            nc.tensor.matmul(
                                at_ps[:], vh[:, i, :], pr[:],
                                start=(n == 0), stop=(n == len(live) - 1),
                                skip_group_check=True)
                            nc.tensor.matmul(
                                dn_ps[:], ones_r[:], pr[:],
                                start=(n == 0), stop=(n == len(live) - 1),
                                skip_group_check=True)
                        rc = smp.tile([1, SQ], F32, tag="rc")
                        nc.vector.reciprocal(rc[:], dn_ps[:])
                        bc_ps = psB1.tile([P, SQ], F32, tag="bc")
                        nc.tensor.matmul(bc_ps[:], ones_f[:], rc[:],
                                         start=True, stop=True)
                        bc_sb = smp.tile([P, SQ], F32, tag="bcs")
                        nc.scalar.activation(
                            bc_sb[:], bc_ps[:], mybir.ActivationFunctionType.Copy)
                        nc.vector.tensor_tensor(
                            attn_t[:, h, qsl], at_ps[:], bc_sb[:], MULT)

            # ---------------- Phase C: output projection ----------------
            with tc.tile_pool(name="wo", bufs=1) as wop, \
                 tc.tile_pool(name="og", bufs=2) as ogp, \
                 tc.tile_pool(name="psC", bufs=4, space="PSUM") as psC:
                wo_t = wop.tile([P, HPG, d], F32R)
                nc.sync.dma_start(
                    wo_t[:], woT[:].rearrange("(ko p) e -> p ko e", p=P).bitcast(F32R))
                for st in range(n_i):
                    og = ogp.tile([P, d], F32, tag="og")
                    for dch in range(d // SQ):
                        po = psC.tile([P, SQ], F32, tag="po")
                        for ct in range(HPG):
                            nc.tensor.matmul(
                                po[:], attn_t[:, ct, st * P:(st + 1) * P],
                                wo_t[:, ct, dch * SQ:(dch + 1) * SQ],
                                start=(ct == 0), stop=(ct == HPG - 1))
                        nc.scalar.activation(
                            og[:, dch * SQ:(dch + 1) * SQ], po[:],
                            mybir.ActivationFunctionType.Copy)
                    nc.sync.dma_start(out[st * P:(st + 1) * P, :], og[:])

    nc.compile()
    return nc


def _prep_host(inputs):
    """Shard + transpose the full inputs into 8 per-core input maps."""
    x = np.asarray(inputs["x"], np.float32)
    wq = np.asarray(inputs["wq"], np.float32)
    wk = np.asarray(inputs["wk"], np.float32)
    wv = np.asarray(inputs["wv"], np.float32)
    wo = np.asarray(inputs["wo"], np.float32)
    cos = np.asarray(inputs["cos"], np.float32)
    sin = np.asarray(inputs["sin"], np.float32)
    mask = np.asarray(inputs["mask"], np.float32)
    start_p = int(inputs["start_p"])

    s = x.shape[1]
    cos_u = cos[start_p:start_p + s]          # (s, HD/2)
    sin_u = sin[start_p:start_p + s]

    # rotate-half channel permutation within each head: [evens, odds]
    perm = np.concatenate(
        [h * HD + np.concatenate([np.arange(0, HD, 2), np.arange(1, HD, 2)])
         for h in range(H)])

    cosP = np.ascontiguousarray(
        np.concatenate([cos_u.T, cos_u.T], axis=0))          # (128, s)
    sinSw = np.ascontiguousarray(
        np.concatenate([sin_u.T, -sin_u.T], axis=0))         # (128, s)

    classes, pats = _classify_mask(mask)
    onesd = np.ones((P, 1), np.float32)

    in_maps = []
    for b in range(B):
        xT = np.ascontiguousarray(x[b].T)
        for g in range(GROUPS):
            rows = perm[g * C:(g + 1) * C]
            in_maps.append({
                "xT": xT,
                "wqT": np.ascontiguousarray(wq[rows, :].T),
                "wkT": np.ascontiguousarray(wk[rows, :].T),
                "wvT": np.ascontiguousarray(wv[g * C:(g + 1) * C, :].T),
                "woT": np.ascontiguousarray(wo[:, g * C:(g + 1) * C].T),
                "cosP": cosP,
                "sinSw": sinSw,
                "mblk": pats,
                "onesd": onesd,
            })
    return in_maps, classes, pats


def _run(inputs, trace=False):
    in_maps, classes, pats = _prep_host(inputs)
    key = (pats.shape[0], tuple(sorted(classes.items())))
    if key not in _PROGRAM_CACHE:
        _PROGRAM_CACHE[key] = _build(classes, pats.shape[0])
    nc = _PROGRAM_CACHE[key]
    res = bass_utils.run_bass_kernel_spmd(
        nc, in_maps, core_ids=list(range(NCORES)), trace=trace)
    out = np.zeros((B, S, D), np.float32)
    for b in range(B):
        acc = res.results[b * GROUPS]["out"].astype(np.float32).copy()
        for g in range(1, GROUPS):
            acc += res.results[b * GROUPS + g]["out"]
        out[b] = acc
    return out, res


def kernel(**inputs):
    out, _ = _run(inputs, trace=False)
    return out


# revision 8
# speedup vs baseline: 1.1172x; 1.1172x over previous
"""Trainium2 Bass kernel for nn_Attention_51307679318359.

Multi-head attention (B=2, S=2048, D=2048, H=16, HD=128) with RoPE and an
additive mask, sharded over 8 NeuronCores as (batch x head-group): each core
computes 1 batch and 4 heads (512 channels), producing a partial output that
the host sums over head-groups.

Per-core dataflow (all activations kept transposed, channels on partitions):
  QT = wqT' @ xT, KT = wkT' @ xT  (rotate-half permuted weights), RoPE applied
  on the PSUM output via DVE; V = xT' @ wvT; spilled to DRAM as f32r (per-head
  tiles so the attention phase can start as soon as head 0 is written).
  Per head: scoresT(sk,sq) = KT_h-slice.T @ QT_h (fp32r, one matmul per
  128x512 block), exp on ACT, multiplicative exp(mask) patterns on the mixed
  blocks, AV + ones-vector denominator accumulated in PSUM, normalization via
  reciprocal + K=1 f32r broadcast matmul + DVE multiply. Out-proj contracts
  the 4 head tiles against woT.
Fully-masked score blocks (causal upper triangle) are skipped based on a
host-side classification of the mask into skip / plain / pattern blocks.
"""

import math

import numpy as np

import concourse.bass as bass
import concourse.mybir as mybir
import concourse.tile as tile
from concourse import bacc
from concourse import bass_utils

F32 = mybir.dt.float32
F32R = mybir.dt.float32r
ADD = mybir.AluOpType.add
MULT = mybir.AluOpType.mult

B, S, D = 2, 2048, 2048
H, HD = 16, 128
NCORES = 8
GROUPS = NCORES // B          # 4 head-groups
HPG = H // GROUPS             # 4 heads per group
C = HPG * HD                  # 512 per-core channels
P = 128
CH_A = 256                    # phase-A s-chunk width
SQ = 512                      # phase-B sq-chunk width
SCALE = 1.0 / math.sqrt(HD)
NEG_THRESH = -1e8             # "masked out" threshold

_PROGRAM_CACHE = {}


def _classify_mask(mask):
    """Classify transposed-mask blocks (sk-tile i x sq-chunk j) and dedupe the
    mixed patterns. Returns (classes, patterns): classes[(j, i)] is
    'skip' | 'plain' | pattern index; patterns is (nblk, 128, SQ) f32 holding
    exp(maskT block) (multiplicative masking applied to the probs after exp).
    """
    maskT = np.ascontiguousarray(mask.T)
    n_j = mask.shape[0] // SQ
    n_i = mask.shape[0] // P
    classes = {}
    patterns = []
    pat_idx = {}

    def add_pattern(blk):
        key = blk.tobytes()
        if key not in pat_idx:
            pat_idx[key] = len(patterns)
            with np.errstate(over='ignore'):
                patterns.append(np.exp(blk.astype(np.float64)).astype(np.float32))
        return pat_idx[key]

    for j in range(n_j):
        for i in range(n_i):
            blk = maskT[i * P:(i + 1) * P, j * SQ:(j + 1) * SQ]
            if np.all(blk == 0.0):
                classes[(j, i)] = 'plain'
            elif np.all(blk <= NEG_THRESH):
                classes[(j, i)] = 'skip'
            else:
                classes[(j, i)] = add_pattern(blk)
    # every sq position must keep at least one live sk tile, else softmax
    # denominators vanish; fall back to no skipping in that degenerate case
    if any(all(classes[(j, i)] == 'skip' for i in range(n_i)) for j in range(n_j)):
        for j in range(n_j):
            for i in range(n_i):
                if classes[(j, i)] == 'skip':
                    blk = maskT[i * P:(i + 1) * P, j * SQ:(j + 1) * SQ]
                    classes[(j, i)] = add_pattern(blk)
    pats = np.stack(patterns, 0).astype(np.float32) if patterns else \
        np.zeros((1, P, SQ), np.float32)
    return classes, pats


def _build(classes, nblk, s=S, d=D):
    """Build + compile the per-core SPMD program."""
    nkt = d // P
    n_j = s // SQ
    n_i = s // P
    n_ja = s // CH_A

    nc = bacc.Bacc("TRN2", target_bir_lowering=False, debug=False)
    xT = nc.dram_tensor("xT", (d, s), F32, kind="ExternalInput")
    wqT = nc.dram_tensor("wqT", (d, C), F32, kind="ExternalInput")
    wkT = nc.dram_tensor("wkT", (d, C), F32, kind="ExternalInput")
    wvT = nc.dram_tensor("wvT", (d, C), F32, kind="ExternalInput")
    woT = nc.dram_tensor("woT", (C, d), F32, kind="ExternalInput")
    cosP = nc.dram_tensor("cosP", (HD, s), F32, kind="ExternalInput")
    sinSw = nc.dram_tensor("sinSw", (HD, s), F32, kind="ExternalInput")
    mblk = nc.dram_tensor("mblk", (nblk, P, SQ), F32, kind="ExternalInput")
    onesd = nc.dram_tensor("onesd", (P, 1), F32, kind="ExternalInput")
    out = nc.dram_tensor("out", (s, d), F32, kind="ExternalOutput")

    with tile.TileContext(nc) as tc:
        with tc.tile_pool(name="dram", bufs=1, space="DRAM") as dram, \
             tc.tile_pool(name="const", bufs=1) as const, \
             tc.tile_pool(name="wo", bufs=1) as wop:
            qh_d = [dram.tile([P, s], F32R, name=f"qh_d{h}") for h in range(HPG)]
            kh_d = [dram.tile([P, s], F32R, name=f"kh_d{h}") for h in range(HPG)]
            vh_d = [dram.tile([s, HD], F32R, name=f"vh_d{h}") for h in range(HPG)]

            ones_r = const.tile([P, 1], F32R)
            nc.sync.dma_start(ones_r[:], onesd[:].bitcast(F32R))
            ones_f = const.tile([1, P], F32R)
            nc.sync.dma_start(ones_f[:],
                              onesd[:].rearrange("a b -> b a").bitcast(F32R))
            mblk_t = const.tile([P, nblk, SQ], F32R)
            nc.sync.dma_start(mblk_t[:], mblk[:].rearrange("n p q -> p n q").bitcast(F32R))
            wo_t = wop.tile([P, HPG, d], F32R)
            nc.sync.dma_start(
                wo_t[:], woT[:].rearrange("(ko p) e -> p ko e", p=P).bitcast(F32R))

            # ---------------- Phase A: QKV projections + RoPE ----------------
            with tc.tile_pool(name="wres", bufs=1) as wres, \
                 tc.tile_pool(name="xc", bufs=2) as xcp, \
                 tc.tile_pool(name="trig", bufs=1) as trig, \
                 tc.tile_pool(name="ptmp", bufs=3) as ptmp, \
                 tc.tile_pool(name="stg", bufs=4) as stg, \
                 tc.tile_pool(name="psA", bufs=2, space="PSUM") as psA:
                cos_t = trig.tile([P, s], F32)
                nc.sync.dma_start(cos_t[:], cosP[:])
                sin_t = trig.tile([P, s], F32)
                nc.sync.dma_start(sin_t[:], sinSw[:])
                wq_t = wres.tile([P, nkt, C], F32R)
                nc.sync.dma_start(
                    wq_t[:], wqT[:].rearrange("(ko p) c -> p ko c", p=P).bitcast(F32R))
                wk_t = wres.tile([P, nkt, C], F32R)
                nc.sync.dma_start(
                    wk_t[:], wkT[:].rearrange("(ko p) c -> p ko c", p=P).bitcast(F32R))
                wv_t = wres.tile([P, nkt, C], F32R)
                nc.sync.dma_start(
                    wv_t[:], wvT[:].rearrange("(ko p) c -> p ko c", p=P).bitcast(F32R))

                for j in range(n_ja):
                    sl = slice(j * CH_A, (j + 1) * CH_A)
                    xc = xcp.tile([P, nkt, CH_A], F32R, tag="xc")
                    nc.sync.dma_start(
                        xc[:],
                        xT[:].rearrange("(ko p) t -> p ko t", p=P)[:, :, sl].bitcast(F32R))
                    for (wt, dst) in ((wq_t, qh_d), (wk_t, kh_d)):
                        for ct in range(HPG):
                            ps = psA.tile([P, CH_A], F32, tag="ps_qk")
                            for k in range(nkt):
                                nc.tensor.matmul(
                                    ps[:], wt[:, k, ct * P:(ct + 1) * P],
                                    xc[:, k, :],
                                    start=(k == 0), stop=(k == nkt - 1))
                            # RoPE (rotate-half layout):
                            #   out_top = x0*cos - x1*sin ; out_bot = x1*cos + x0*sin
                            t1 = ptmp.tile([P, CH_A], F32, tag="t1")
                            nc.vector.tensor_tensor(t1[:], ps[:], cos_t[:, sl], MULT)
                            t2 = ptmp.tile([P, CH_A], F32, tag="t2")
                            nc.vector.tensor_tensor(
                                t2[0:64, :], ps[64:128, :], sin_t[64:128, sl], MULT)
                            nc.vector.tensor_tensor(
                                t2[64:128, :], ps[0:64, :], sin_t[0:64, sl], MULT)
                            ro = stg.tile([P, CH_A], F32R, tag="ro")
                            nc.vector.tensor_tensor(ro[:], t1[:], t2[:], ADD)
                            nc.sync.dma_start(dst[ct][:, sl], ro[:])
                    for st2 in range(CH_A // P):
                        st = (j * CH_A) // P + st2
                        psv = psA.tile([P, C], F32, tag="ps_v")
                        for k in range(nkt):
                            nc.tensor.matmul(
                                psv[:], xc[:, k, st2 * P:(st2 + 1) * P],
                                wv_t[:, k, :],
                                start=(k == 0), stop=(k == nkt - 1))
                        vo = stg.tile([P, C], F32R, tag="vo")
                        nc.vector.tensor_copy(vo[:], psv[:])
                        for h in range(HPG):
                            nc.sync.dma_start(
                                vh_d[h][st * P:(st + 1) * P, :],
                                vo[:, h * HD:(h + 1) * HD])

            # ---------------- Phase B: attention per head ----------------
            with tc.tile_pool(name="attn", bufs=1) as attnp:
              attn_t = attnp.tile([P, HPG, s], F32R)
              with tc.tile_pool(name="hq", bufs=2) as hqp, \
                 tc.tile_pool(name="pr", bufs=6) as prp, \
                 tc.tile_pool(name="sm", bufs=2) as smp, \
                 tc.tile_pool(name="psS", bufs=3, space="PSUM") as psS, \
                 tc.tile_pool(name="psB", bufs=2, space="PSUM") as psB, \
                 tc.tile_pool(name="psB1", bufs=1, space="PSUM") as psB1:
                for h in range(HPG):
                    qh = hqp.tile([P, s], F32R, tag="qh")
                    nc.sync.dma_start(qh[:], qh_d[h][:])
                    kh = hqp.tile([P, s], F32R, tag="kh")
                    nc.sync.dma_start(kh[:], kh_d[h][:])
                    vh = hqp.tile([P, s // P, HD], F32R, tag="vh")
                    nc.sync.dma_start(
                        vh[:], vh_d[h][:].rearrange("(ko p) c -> p ko c", p=P))
                    for jq in range(n_j):
                        live = [i for i in range(n_i) if classes[(jq, i)] != 'skip']
                        qsl = slice(jq * SQ, (jq + 1) * SQ)
                        at_ps = psB.tile([P, SQ], F32, tag="at")
                        dn_ps = psB.tile([1, SQ], F32, tag="dn")
                        for n, i in enumerate(live):
                            sc = psS.tile([P, SQ], F32, tag="sc")
                            nc.tensor.matmul(
                                sc[:], kh[:, i * P:(i + 1) * P], qh[:, qsl],
                                start=True, stop=True)
                            pr = prp.tile([P, SQ], F32R, tag="pr")
                            nc.scalar.activation(
                                pr[:], sc[:], mybir.ActivationFunctionType.Exp,
                                scale=SCALE)
                            cls = classes[(jq, i)]
                            if isinstance(cls, int):
                                nc.vector.tensor_tensor(
                                    pr[:], pr[:], mblk_t[:, cls, :], MULT)
                            nc.tensor.matmul(
                                at_ps[:], vh[:, i, :], pr[:],
                                start=(n == 0), stop=(n == len(live) - 1),
                                skip_group_check=True)
                            nc.tensor.matmul(
                                dn_ps[:], ones_r[:], pr[:],
                                start=(n == 0), stop=(n == len(live) - 1),
                                skip_group_check=True)
                        rc = smp.tile([1, SQ], F32R, tag="rc")
                        with nc.allow_low_precision(reason="f32r is bitwise f32"):
                            nc.vector.reciprocal(rc[:], dn_ps[:])
                        bc_ps = psB1.tile([P, SQ], F32, tag="bc")
                        nc.tensor.matmul(bc_ps[:], ones_f[:], rc[:],
                                         start=True, stop=True)
                        bc_sb = smp.tile([P, SQ], F32, tag="bcs")
                        nc.scalar.activation(
                            bc_sb[:], bc_ps[:], mybir.ActivationFunctionType.Copy)
                        nc.vector.tensor_tensor(
                            attn_t[:, h, qsl], at_ps[:], bc_sb[:], MULT)

              # ---------------- Phase C: output projection ----------------
              with tc.tile_pool(name="og", bufs=2) as ogp, \
                   tc.tile_pool(name="psC", bufs=4, space="PSUM") as psC:
                  for st in range(n_i):
                      og = ogp.tile([P, d], F32, tag="og")
                      for dch in range(d // SQ):
                          po = psC.tile([P, SQ], F32, tag="po")
                          for ct in range(HPG):
                              nc.tensor.matmul(
                                  po[:], attn_t[:, ct, st * P:(st + 1) * P],
                                  wo_t[:, ct, dch * SQ:(dch + 1) * SQ],
                                  start=(ct == 0), stop=(ct == HPG - 1))
                          nc.scalar.activation(
                              og[:, dch * SQ:(dch + 1) * SQ], po[:],
                              mybir.ActivationFunctionType.Copy)
                      nc.sync.dma_start(out[st * P:(st + 1) * P, :], og[:])

    nc.compile()
    return nc


def _prep_host(inputs):
    """Shard + transpose the full inputs into 8 per-core input maps."""
    x = np.asarray(inputs["x"], np.float32)
    wq = np.asarray(inputs["wq"], np.float32)
    wk = np.asarray(inputs["wk"], np.float32)
    wv = np.asarray(inputs["wv"], np.float32)
    wo = np.asarray(inputs["wo"], np.float32)
    cos = np.asarray(inputs["cos"], np.float32)
    sin = np.asarray(inputs["sin"], np.float32)
    mask = np.asarray(inputs["mask"], np.float32)
    start_p = int(inputs["start_p"])

    s = x.shape[1]
    cos_u = cos[start_p:start_p + s]          # (s, HD/2)
    sin_u = sin[start_p:start_p + s]

    # rotate-half channel permutation within each head: [evens, odds]
    perm = np.concatenate(
        [h * HD + np.concatenate([np.arange(0, HD, 2), np.arange(1, HD, 2)])
         for h in range(H)])

    cosP = np.ascontiguousarray(
        np.concatenate([cos_u.T, cos_u.T], axis=0))          # (128, s)
    sinSw = np.ascontiguousarray(
        np.concatenate([sin_u.T, -sin_u.T], axis=0))         # (128, s)

    classes, pats = _classify_mask(mask)
    onesd = np.ones((P, 1), np.float32)

    in_maps = []
    for b in range(B):
        xT = np.ascontiguousarray(x[b].T)
        for g in range(GROUPS):
            rows = perm[g * C:(g + 1) * C]
            in_maps.append({
                "xT": xT,
                "wqT": np.ascontiguousarray(wq[rows, :].T),
                "wkT": np.ascontiguousarray(wk[rows, :].T),
                "wvT": np.ascontiguousarray(wv[g * C:(g + 1) * C, :].T),
                "woT": np.ascontiguousarray(wo[:, g * C:(g + 1) * C].T),
                "cosP": cosP,
                "sinSw": sinSw,
                "mblk": pats,
                "onesd": onesd,
            })
    return in_maps, classes, pats


def _run(inputs, trace=False):
    in_maps, classes, pats = _prep_host(inputs)
    key = (pats.shape[0], tuple(sorted(classes.items())))
    if key not in _PROGRAM_CACHE:
        _PROGRAM_CACHE[key] = _build(classes, pats.shape[0])
    nc = _PROGRAM_CACHE[key]
    res = bass_utils.run_bass_kernel_spmd(
        nc, in_maps, core_ids=list(range(NCORES)), trace=trace)
    out = np.zeros((B, S, D), np.float32)
    for b in range(B):
        acc = res.results[b * GROUPS]["out"].astype(np.float32).copy()
        for g in range(1, GROUPS):
            acc += res.results[b * GROUPS + g]["out"]
        out[b] = acc
    return out, res


def kernel(**inputs):
    out, _ = _run(inputs, trace=False)
    return out


# revision 11
# speedup vs baseline: 1.2864x; 1.1514x over previous
"""Trainium2 Bass kernel for nn_Attention_51307679318359.

Multi-head attention (B=2, S=2048, D=2048, H=16, HD=128) with RoPE and an
additive mask, sharded over 8 NeuronCores as (batch x head-group): each core
computes 1 batch and 4 heads (512 channels), producing a partial output that
the host sums over head-groups.

Per-core dataflow (all activations kept transposed, channels on partitions):
  QT = wqT' @ xT, KT = wkT' @ xT  (rotate-half permuted weights), RoPE applied
  on the PSUM output via DVE; V = xT' @ wvT; spilled to DRAM as f32r (per-head
  tiles so the attention phase can start as soon as head 0 is written).
  Per head: scoresT(sk,sq) = KT_h-slice.T @ QT_h (fp32r, one matmul per
  128x512 block), exp on ACT, multiplicative exp(mask) patterns on the mixed
  blocks, AV + ones-vector denominator accumulated in PSUM, normalization via
  reciprocal + K=1 f32r broadcast matmul + DVE multiply. Out-proj contracts
  the 4 head tiles against woT.
Fully-masked score blocks (causal upper triangle) are skipped based on a
host-side classification of the mask into skip / plain / pattern blocks.
"""

import math

import numpy as np

import concourse.bass as bass
import concourse.mybir as mybir
import concourse.tile as tile
from concourse import bacc
from concourse import bass_utils

F32 = mybir.dt.float32
F32R = mybir.dt.float32r
ADD = mybir.AluOpType.add
MULT = mybir.AluOpType.mult

B, S, D = 2, 2048, 2048
H, HD = 16, 128
NCORES = 8
GROUPS = NCORES // B          # 4 head-groups
HPG = H // GROUPS             # 4 heads per group
C = HPG * HD                  # 512 per-core channels
P = 128
CH_A = 256                    # phase-A s-chunk width
SQ = 512                      # phase-B sq-chunk width
SCALE = 1.0 / math.sqrt(HD)
NEG_THRESH = -1e8             # "masked out" threshold

_PROGRAM_CACHE = {}


def _classify_mask(mask):
    """Classify transposed-mask blocks (sk-tile i x sq-chunk j) and dedupe the
    mixed patterns. Returns (classes, patterns): classes[(j, i)] is
    'skip' | 'plain' | pattern index; patterns is (nblk, 128, SQ) f32 holding
    exp(maskT block) (multiplicative masking applied to the probs after exp).
    """
    maskT = np.ascontiguousarray(mask.T)
    n_j = mask.shape[0] // SQ
    n_i = mask.shape[0] // P
    classes = {}
    patterns = []
    pat_idx = {}

    def add_pattern(blk):
        key = blk.tobytes()
        if key not in pat_idx:
            pat_idx[key] = len(patterns)
            with np.errstate(over='ignore'):
                patterns.append(np.exp(blk.astype(np.float64)).astype(np.float32))
        return pat_idx[key]

    for j in range(n_j):
        for i in range(n_i):
            blk = maskT[i * P:(i + 1) * P, j * SQ:(j + 1) * SQ]
            if np.all(blk == 0.0):
                classes[(j, i)] = 'plain'
            elif np.all(blk <= NEG_THRESH):
                classes[(j, i)] = 'skip'
            else:
                classes[(j, i)] = add_pattern(blk)
    # every sq position must keep at least one live sk tile, else softmax
    # denominators vanish; fall back to no skipping in that degenerate case
    if any(all(classes[(j, i)] == 'skip' for i in range(n_i)) for j in range(n_j)):
        for j in range(n_j):
            for i in range(n_i):
                if classes[(j, i)] == 'skip':
                    blk = maskT[i * P:(i + 1) * P, j * SQ:(j + 1) * SQ]
                    classes[(j, i)] = add_pattern(blk)
    pats = np.stack(patterns, 0).astype(np.float32) if patterns else \
        np.zeros((1, P, SQ), np.float32)
    return classes, pats


def _build(classes, nblk, s=S, d=D):
    """Build + compile the per-core SPMD program."""
    nkt = d // P
    n_j = s // SQ
    n_i = s // P
    n_ja = s // CH_A

    nc = bacc.Bacc("TRN2", target_bir_lowering=False, debug=False)
    xT = nc.dram_tensor("xT", (d, s), F32, kind="ExternalInput")
    wqT = nc.dram_tensor("wqT", (d, C), F32, kind="ExternalInput")
    wkT = nc.dram_tensor("wkT", (d, C), F32, kind="ExternalInput")
    wvT = nc.dram_tensor("wvT", (d, C), F32, kind="ExternalInput")
    woT = nc.dram_tensor("woT", (C, d), F32, kind="ExternalInput")
    cosP = nc.dram_tensor("cosP", (HD, s), F32, kind="ExternalInput")
    sinSw = nc.dram_tensor("sinSw", (HD, s), F32, kind="ExternalInput")
    mblk = nc.dram_tensor("mblk", (nblk, P, SQ), F32, kind="ExternalInput")
    onesd = nc.dram_tensor("onesd", (P, 1), F32, kind="ExternalInput")
    out = nc.dram_tensor("out", (s, d), F32, kind="ExternalOutput")

    with tile.TileContext(nc) as tc:
        with tc.tile_pool(name="dram", bufs=1, space="DRAM") as dram, \
             tc.tile_pool(name="const", bufs=1) as const:
            qh_d = [dram.tile([P, s], F32R, name=f"qh_d{h}") for h in range(HPG)]
            kh_d = [dram.tile([P, s], F32R, name=f"kh_d{h}") for h in range(HPG)]
            vh_d = [dram.tile([s, HD], F32R, name=f"vh_d{h}") for h in range(HPG)]

            ones_r = const.tile([P, 1], F32R)
            nc.sync.dma_start(ones_r[:], onesd[:].bitcast(F32R))
            ones_f = const.tile([1, P], F32R)
            nc.sync.dma_start(ones_f[:],
                              onesd[:].rearrange("a b -> b a").bitcast(F32R))
            mblk_t = const.tile([P, nblk, SQ], F32R)
            nc.sync.dma_start(mblk_t[:], mblk[:].rearrange("n p q -> p n q").bitcast(F32R))
            q0_sb = const.tile([P, s], F32R)
            k0_sb = const.tile([P, s], F32R)
            v0_sb = const.tile([P, s // P, HD], F32R)

            # ---------------- Phase A: QKV projections + RoPE ----------------
            with tc.tile_pool(name="wres", bufs=1) as wres, \
                 tc.tile_pool(name="xc", bufs=2) as xcp, \
                 tc.tile_pool(name="trig", bufs=1) as trig, \
                 tc.tile_pool(name="ptmp", bufs=2) as ptmp, \
                 tc.tile_pool(name="stg", bufs=3) as stg, \
                 tc.tile_pool(name="psA", bufs=2, space="PSUM") as psA:
                cos_t = trig.tile([P, s], F32)
                nc.sync.dma_start(cos_t[:], cosP[:])
                sin_t = trig.tile([P, s], F32)
                nc.sync.dma_start(sin_t[:], sinSw[:])
                wq_t = wres.tile([P, nkt, C], F32R)
                nc.sync.dma_start(
                    wq_t[:], wqT[:].rearrange("(ko p) c -> p ko c", p=P).bitcast(F32R))
                wk_t = wres.tile([P, nkt, C], F32R)
                nc.sync.dma_start(
                    wk_t[:], wkT[:].rearrange("(ko p) c -> p ko c", p=P).bitcast(F32R))
                wv_t = wres.tile([P, nkt, C], F32R)
                nc.sync.dma_start(
                    wv_t[:], wvT[:].rearrange("(ko p) c -> p ko c", p=P).bitcast(F32R))

                for j in range(n_ja):
                    sl = slice(j * CH_A, (j + 1) * CH_A)
                    xc = xcp.tile([P, nkt, CH_A], F32R, tag="xc")
                    nc.sync.dma_start(
                        xc[:],
                        xT[:].rearrange("(ko p) t -> p ko t", p=P)[:, :, sl].bitcast(F32R))
                    for (wt, dst, sb0) in ((wq_t, qh_d, q0_sb), (wk_t, kh_d, k0_sb)):
                        for ct in range(HPG):
                            ps = psA.tile([P, CH_A], F32, tag="ps_qk")
                            for k in range(nkt):
                                nc.tensor.matmul(
                                    ps[:], wt[:, k, ct * P:(ct + 1) * P],
                                    xc[:, k, :],
                                    start=(k == 0), stop=(k == nkt - 1))
                            # RoPE (rotate-half layout):
                            #   out_top = x0*cos - x1*sin ; out_bot = x1*cos + x0*sin
                            t1 = ptmp.tile([P, CH_A], F32, tag="t1")
                            nc.vector.tensor_tensor(t1[:], ps[:], cos_t[:, sl], MULT)
                            t2 = ptmp.tile([P, CH_A], F32, tag="t2")
                            nc.vector.tensor_tensor(
                                t2[0:64, :], ps[64:128, :], sin_t[64:128, sl], MULT)
                            nc.vector.tensor_tensor(
                                t2[64:128, :], ps[0:64, :], sin_t[0:64, sl], MULT)
                            if ct == 0:
                                # head 0 stays SBUF-resident across the A->B boundary
                                nc.vector.tensor_tensor(
                                    sb0[:, sl], t1[:], t2[:], ADD)
                            else:
                                ro = stg.tile([P, CH_A], F32R, tag="ro")
                                nc.vector.tensor_tensor(ro[:], t1[:], t2[:], ADD)
                                nc.sync.dma_start(dst[ct][:, sl], ro[:])
                    for st2 in range(CH_A // P):
                        st = (j * CH_A) // P + st2
                        psv = psA.tile([P, C], F32, tag="ps_v")
                        for k in range(nkt):
                            nc.tensor.matmul(
                                psv[:], xc[:, k, st2 * P:(st2 + 1) * P],
                                wv_t[:, k, :],
                                start=(k == 0), stop=(k == nkt - 1))
                        nc.vector.tensor_copy(v0_sb[:, st, :], psv[:, 0:HD])
                        vo = stg.tile([P, C - HD], F32R, tag="vo")
                        nc.vector.tensor_copy(vo[:], psv[:, HD:])
                        for h in range(1, HPG):
                            nc.sync.dma_start(
                                vh_d[h][st * P:(st + 1) * P, :],
                                vo[:, (h - 1) * HD:h * HD])

            # ---------------- Phase B: attention per head ----------------
            with tc.tile_pool(name="attn", bufs=1) as attnp:
              attn_t = attnp.tile([P, HPG, s], F32R)
              wo_t = attnp.tile([P, HPG, d], F32R)
              nc.sync.dma_start(
                  wo_t[:], woT[:].rearrange("(ko p) e -> p ko e", p=P).bitcast(F32R))
              with tc.tile_pool(name="hq", bufs=2) as hqp, \
                 tc.tile_pool(name="pr", bufs=6) as prp, \
                 tc.tile_pool(name="rcp", bufs=HPG * n_j) as rcp, \
                 tc.tile_pool(name="sm", bufs=3) as smp, \
                 tc.tile_pool(name="psS", bufs=3, space="PSUM") as psS, \
                 tc.tile_pool(name="psB", bufs=2, space="PSUM") as psB, \
                 tc.tile_pool(name="psB1", bufs=1, space="PSUM") as psB1:
                rcs = {}
                for h in range(HPG):
                    if h == 0:
                        qh, kh, vh = q0_sb, k0_sb, v0_sb
                    else:
                        qh = hqp.tile([P, s], F32R, tag="qh")
                        nc.sync.dma_start(qh[:], qh_d[h][:])
                        kh = hqp.tile([P, s], F32R, tag="kh")
                        nc.sync.dma_start(kh[:], kh_d[h][:])
                        vh = hqp.tile([P, s // P, HD], F32R, tag="vh")
                        nc.sync.dma_start(
                            vh[:], vh_d[h][:].rearrange("(ko p) c -> p ko c", p=P))
                    for jq in range(n_j):
                        live = [i for i in range(n_i) if classes[(jq, i)] != 'skip']
                        qsl = slice(jq * SQ, (jq + 1) * SQ)
                        at_ps = psB.tile([P, SQ], F32, tag="at")
                        dn_ps = psB.tile([1, SQ], F32, tag="dn")
                        for n, i in enumerate(live):
                            sc = psS.tile([P, SQ], F32, tag="sc")
                            nc.tensor.matmul(
                                sc[:], kh[:, i * P:(i + 1) * P], qh[:, qsl],
                                start=True, stop=True)
                            pr = prp.tile([P, SQ], F32R, tag="pr")
                            nc.scalar.activation(
                                pr[:], sc[:], mybir.ActivationFunctionType.Exp,
                                scale=SCALE)
                            cls = classes[(jq, i)]
                            if isinstance(cls, int):
                                nc.vector.tensor_tensor(
                                    pr[:], pr[:], mblk_t[:, cls, :], MULT)
                            nc.tensor.matmul(
                                at_ps[:], vh[:, i, :], pr[:],
                                start=(n == 0), stop=(n == len(live) - 1),
                                skip_group_check=True)
                            nc.tensor.matmul(
                                dn_ps[:], ones_r[:], pr[:],
                                start=(n == 0), stop=(n == len(live) - 1),
                                skip_group_check=True)
                        # stash the unnormalized row block + its denominators;
                        # normalization is deferred so the PE never waits on it
                        nc.scalar.activation(
                            attn_t[:, h, qsl], at_ps[:],
                            mybir.ActivationFunctionType.Copy)
                        rc = rcp.tile([1, SQ], F32R, tag="rc")
                        with nc.allow_low_precision(reason="f32r is bitwise f32"):
                            nc.vector.reciprocal(rc[:], dn_ps[:])
                        rcs[(h, jq)] = rc
                for (h, jq), rc in rcs.items():
                    qsl = slice(jq * SQ, (jq + 1) * SQ)
                    bc_ps = psB1.tile([P, SQ], F32, tag="bc")
                    nc.tensor.matmul(bc_ps[:], ones_f[:], rc[:],
                                     start=True, stop=True)
                    bc_sb = smp.tile([P, SQ], F32, tag="bcs")
                    nc.scalar.activation(
                        bc_sb[:], bc_ps[:], mybir.ActivationFunctionType.Copy)
                    nc.vector.tensor_tensor(
                        attn_t[:, h, qsl], attn_t[:, h, qsl], bc_sb[:], MULT)

              # ---------------- Phase C: output projection ----------------
              with tc.tile_pool(name="og", bufs=2) as ogp, \
                   tc.tile_pool(name="psC", bufs=4, space="PSUM") as psC:
                  for st in range(n_i):
                      og = ogp.tile([P, d], F32, tag="og")
                      for dch in range(d // SQ):
                          po = psC.tile([P, SQ], F32, tag="po")
                          for ct in range(HPG):
                              nc.tensor.matmul(
                                  po[:], attn_t[:, ct, st * P:(st + 1) * P],
                                  wo_t[:, ct, dch * SQ:(dch + 1) * SQ],
                                  start=(ct == 0), stop=(ct == HPG - 1))
                          nc.scalar.activation(
                              og[:, dch * SQ:(dch + 1) * SQ], po[:],
                              mybir.ActivationFunctionType.Copy)
                      nc.sync.dma_start(out[st * P:(st + 1) * P, :], og[:])

    nc.compile()
    return nc


def _prep_host(inputs):
    """Shard + transpose the full inputs into 8 per-core input maps."""
    x = np.asarray(inputs["x"], np.float32)
    wq = np.asarray(inputs["wq"], np.float32)
    wk = np.asarray(inputs["wk"], np.float32)
    wv = np.asarray(inputs["wv"], np.float32)
    wo = np.asarray(inputs["wo"], np.float32)
    cos = np.asarray(inputs["cos"], np.float32)
    sin = np.asarray(inputs["sin"], np.float32)
    mask = np.asarray(inputs["mask"], np.float32)
    start_p = int(inputs["start_p"])

    s = x.shape[1]
    cos_u = cos[start_p:start_p + s]          # (s, HD/2)
    sin_u = sin[start_p:start_p + s]

    # rotate-half channel permutation within each head: [evens, odds]
    perm = np.concatenate(
        [h * HD + np.concatenate([np.arange(0, HD, 2), np.arange(1, HD, 2)])
         for h in range(H)])

    cosP = np.ascontiguousarray(
        np.concatenate([cos_u.T, cos_u.T], axis=0))          # (128, s)
    sinSw = np.ascontiguousarray(
        np.concatenate([sin_u.T, -sin_u.T], axis=0))         # (128, s)

    classes, pats = _classify_mask(mask)
    onesd = np.ones((P, 1), np.float32)

    in_maps = []
    for b in range(B):
        xT = np.ascontiguousarray(x[b].T)
        for g in range(GROUPS):
            rows = perm[g * C:(g + 1) * C]
            in_maps.append({
                "xT": xT,
                "wqT": np.ascontiguousarray(wq[rows, :].T),
                "wkT": np.ascontiguousarray(wk[rows, :].T),
                "wvT": np.ascontiguousarray(wv[g * C:(g + 1) * C, :].T),
                "woT": np.ascontiguousarray(wo[:, g * C:(g + 1) * C].T),
                "cosP": cosP,
                "sinSw": sinSw,
                "mblk": pats,
                "onesd": onesd,
            })
    return in_maps, classes, pats


def _run(inputs, trace=False):
    in_maps, classes, pats = _prep_host(inputs)
    key = (pats.shape[0], tuple(sorted(classes.items())))
    if key not in _PROGRAM_CACHE:
        _PROGRAM_CACHE[key] = _build(classes, pats.shape[0])
    nc = _PROGRAM_CACHE[key]
    res = bass_utils.run_bass_kernel_spmd(
        nc, in_maps, core_ids=list(range(NCORES)), trace=trace)
    out = np.zeros((B, S, D), np.float32)
    for b in range(B):
        acc = res.results[b * GROUPS]["out"].astype(np.float32).copy()
        for g in range(1, GROUPS):
            acc += res.results[b * GROUPS + g]["out"]
        out[b] = acc
    return out, res


def kernel(**inputs):
    out, _ = _run(inputs, trace=False)
    return out


# revision 12
# speedup vs baseline: 1.2882x; 1.0014x over previous
"""Trainium2 Bass kernel for nn_Attention_51307679318359.

Multi-head attention (B=2, S=2048, D=2048, H=16, HD=128) with RoPE and an
additive mask, sharded over 8 NeuronCores as (batch x head-group): each core
computes 1 batch and 4 heads (512 channels), producing a partial output that
the host sums over head-groups.

Per-core dataflow (all activations kept transposed, channels on partitions):
  QT = wqT' @ xT, KT = wkT' @ xT  (rotate-half permuted weights), RoPE applied
  on the PSUM output via DVE; V = xT' @ wvT; spilled to DRAM as f32r (per-head
  tiles so the attention phase can start as soon as head 0 is written).
  Per head: scoresT(sk,sq) = KT_h-slice.T @ QT_h (fp32r, one matmul per
  128x512 block), exp on ACT, multiplicative exp(mask) patterns on the mixed
  blocks, AV + ones-vector denominator accumulated in PSUM, normalization via
  reciprocal + K=1 f32r broadcast matmul + DVE multiply. Out-proj contracts
  the 4 head tiles against woT.
Fully-masked score blocks (causal upper triangle) are skipped based on a
host-side classification of the mask into skip / plain / pattern blocks.
"""

import math

import numpy as np

import concourse.bass as bass
import concourse.mybir as mybir
import concourse.tile as tile
from concourse import bacc
from concourse import bass_utils

F32 = mybir.dt.float32
F32R = mybir.dt.float32r
ADD = mybir.AluOpType.add
MULT = mybir.AluOpType.mult

B, S, D = 2, 2048, 2048
H, HD = 16, 128
NCORES = 8
GROUPS = NCORES // B          # 4 head-groups
HPG = H // GROUPS             # 4 heads per group
C = HPG * HD                  # 512 per-core channels
P = 128
CH_A = 256                    # phase-A s-chunk width
SQ = 512                      # phase-B sq-chunk width
SCALE = 1.0 / math.sqrt(HD)
NEG_THRESH = -1e8             # "masked out" threshold

_PROGRAM_CACHE = {}


def _pre_w(wT):
    """(d, c) row-major -> (128, d//128, c) partition-major contiguous."""
    d, c = wT.shape
    return np.ascontiguousarray(wT.reshape(d // P, P, c).transpose(1, 0, 2))


def _pre_x(xT):
    """(d, s) -> (s//CH_A, 128, d//128, CH_A) chunk-major contiguous."""
    d, s = xT.shape
    return np.ascontiguousarray(
        xT.reshape(d // P, P, s // CH_A, CH_A).transpose(2, 1, 0, 3))


def _classify_mask(mask):
    """Classify transposed-mask blocks (sk-tile i x sq-chunk j) and dedupe the
    mixed patterns. Returns (classes, patterns): classes[(j, i)] is
    'skip' | 'plain' | pattern index; patterns is (nblk, 128, SQ) f32 holding
    exp(maskT block) (multiplicative masking applied to the probs after exp).
    """
    maskT = np.ascontiguousarray(mask.T)
    n_j = mask.shape[0] // SQ
    n_i = mask.shape[0] // P
    classes = {}
    patterns = []
    pat_idx = {}

    def add_pattern(blk):
        key = blk.tobytes()
        if key not in pat_idx:
            pat_idx[key] = len(patterns)
            with np.errstate(over='ignore'):
                patterns.append(np.exp(blk.astype(np.float64)).astype(np.float32))
        return pat_idx[key]

    for j in range(n_j):
        for i in range(n_i):
            blk = maskT[i * P:(i + 1) * P, j * SQ:(j + 1) * SQ]
            if np.all(blk == 0.0):
                classes[(j, i)] = 'plain'
            elif np.all(blk <= NEG_THRESH):
                classes[(j, i)] = 'skip'
            else:
                classes[(j, i)] = add_pattern(blk)
    # every sq position must keep at least one live sk tile, else softmax
    # denominators vanish; fall back to no skipping in that degenerate case
    if any(all(classes[(j, i)] == 'skip' for i in range(n_i)) for j in range(n_j)):
        for j in range(n_j):
            for i in range(n_i):
                if classes[(j, i)] == 'skip':
                    blk = maskT[i * P:(i + 1) * P, j * SQ:(j + 1) * SQ]
                    classes[(j, i)] = add_pattern(blk)
    pats = np.stack(patterns, 0).astype(np.float32) if patterns else \
        np.zeros((1, P, SQ), np.float32)
    return classes, pats


def _build(classes, nblk, s=S, d=D):
    """Build + compile the per-core SPMD program."""
    nkt = d // P
    n_j = s // SQ
    n_i = s // P
    n_ja = s // CH_A

    nc = bacc.Bacc("TRN2", target_bir_lowering=False, debug=False)
    xT = nc.dram_tensor("xT", (n_ja, P, nkt, CH_A), F32, kind="ExternalInput")
    wqT = nc.dram_tensor("wqT", (P, nkt, C), F32, kind="ExternalInput")
    wkT = nc.dram_tensor("wkT", (P, nkt, C), F32, kind="ExternalInput")
    wvT = nc.dram_tensor("wvT", (P, nkt, C), F32, kind="ExternalInput")
    woT = nc.dram_tensor("woT", (P, HPG, d), F32, kind="ExternalInput")
    cosP = nc.dram_tensor("cosP", (HD, s), F32, kind="ExternalInput")
    sinSw = nc.dram_tensor("sinSw", (HD, s), F32, kind="ExternalInput")
    mblk = nc.dram_tensor("mblk", (P, nblk, SQ), F32, kind="ExternalInput")
    onesd = nc.dram_tensor("onesd", (P, 1), F32, kind="ExternalInput")
    out = nc.dram_tensor("out", (s, d), F32, kind="ExternalOutput")

    with tile.TileContext(nc) as tc:
        with tc.tile_pool(name="dram", bufs=1, space="DRAM") as dram, \
             tc.tile_pool(name="const", bufs=1) as const:
            qh_d = [dram.tile([P, s], F32R, name=f"qh_d{h}") for h in range(HPG)]
            kh_d = [dram.tile([P, s], F32R, name=f"kh_d{h}") for h in range(HPG)]
            vh_d = [dram.tile([P, s // P, HD], F32R, name=f"vh_d{h}")
                    for h in range(HPG)]

            ones_r = const.tile([P, 1], F32R)
            nc.scalar.dma_start(ones_r[:], onesd[:].bitcast(F32R))
            ones_f = const.tile([1, P], F32R)
            nc.scalar.dma_start(ones_f[:],
                                onesd[:].rearrange("a b -> b a").bitcast(F32R))
            mblk_t = const.tile([P, nblk, SQ], F32R)
            nc.scalar.dma_start(mblk_t[:], mblk[:].bitcast(F32R))
            q0_sb = const.tile([P, s], F32R)
            k0_sb = const.tile([P, s], F32R)
            v0_sb = const.tile([P, s // P, HD], F32R)

            # ---------------- Phase A: QKV projections + RoPE ----------------
            with tc.tile_pool(name="wres", bufs=1) as wres, \
                 tc.tile_pool(name="xc", bufs=2) as xcp, \
                 tc.tile_pool(name="trig", bufs=1) as trig, \
                 tc.tile_pool(name="ptmp", bufs=2) as ptmp, \
                 tc.tile_pool(name="stg", bufs=3) as stg, \
                 tc.tile_pool(name="psA", bufs=2, space="PSUM") as psA:
                wq_t = wres.tile([P, nkt, C], F32R)
                nc.sync.dma_start(wq_t[:], wqT[:].bitcast(F32R))
                cos_t = trig.tile([P, s], F32)
                nc.scalar.dma_start(cos_t[:], cosP[:])
                sin_t = trig.tile([P, s], F32)
                nc.scalar.dma_start(sin_t[:], sinSw[:])
                wk_t = wres.tile([P, nkt, C], F32R)
                nc.sync.dma_start(wk_t[:], wkT[:].bitcast(F32R))
                wv_t = wres.tile([P, nkt, C], F32R)
                nc.sync.dma_start(wv_t[:], wvT[:].bitcast(F32R))

                for j in range(n_ja):
                    sl = slice(j * CH_A, (j + 1) * CH_A)
                    xc = xcp.tile([P, nkt, CH_A], F32R, tag="xc")
                    nc.scalar.dma_start(xc[:], xT[j].bitcast(F32R))
                    for (wt, dst, sb0) in ((wq_t, qh_d, q0_sb), (wk_t, kh_d, k0_sb)):
                        for ct in range(HPG):
                            ps = psA.tile([P, CH_A], F32, tag="ps_qk")
                            for k in range(nkt):
                                nc.tensor.matmul(
                                    ps[:], wt[:, k, ct * P:(ct + 1) * P],
                                    xc[:, k, :],
                                    start=(k == 0), stop=(k == nkt - 1))
                            # RoPE (rotate-half layout):
                            #   out_top = x0*cos - x1*sin ; out_bot = x1*cos + x0*sin
                            t1 = ptmp.tile([P, CH_A], F32, tag="t1")
                            nc.vector.tensor_tensor(t1[:], ps[:], cos_t[:, sl], MULT)
                            t2 = ptmp.tile([P, CH_A], F32, tag="t2")
                            nc.vector.tensor_tensor(
                                t2[0:64, :], ps[64:128, :], sin_t[64:128, sl], MULT)
                            nc.vector.tensor_tensor(
                                t2[64:128, :], ps[0:64, :], sin_t[0:64, sl], MULT)
                            if ct == 0:
                                # head 0 stays SBUF-resident across the A->B boundary
                                nc.vector.tensor_tensor(
                                    sb0[:, sl], t1[:], t2[:], ADD)
                            else:
                                ro = stg.tile([P, CH_A], F32R, tag="ro")
                                nc.vector.tensor_tensor(ro[:], t1[:], t2[:], ADD)
                                nc.sync.dma_start(dst[ct][:, sl], ro[:])
                    for st2 in range(CH_A // P):
                        st = (j * CH_A) // P + st2
                        psv = psA.tile([P, C], F32, tag="ps_v")
                        for k in range(nkt):
                            nc.tensor.matmul(
                                psv[:], xc[:, k, st2 * P:(st2 + 1) * P],
                                wv_t[:, k, :],
                                start=(k == 0), stop=(k == nkt - 1))
                        nc.vector.tensor_copy(v0_sb[:, st, :], psv[:, 0:HD])
                        vo = stg.tile([P, C - HD], F32R, tag="vo")
                        nc.vector.tensor_copy(vo[:], psv[:, HD:])
                        for h in range(1, HPG):
                            nc.sync.dma_start(
                                vh_d[h][:, st, :], vo[:, (h - 1) * HD:h * HD])

            # ---------------- Phase B: attention per head ----------------
            with tc.tile_pool(name="attn", bufs=1) as attnp:
              attn_t = attnp.tile([P, HPG, s], F32R)
              wo_t = attnp.tile([P, HPG, d], F32R)
              nc.sync.dma_start(wo_t[:], woT[:].bitcast(F32R))
              with tc.tile_pool(name="hq", bufs=2) as hqp, \
                 tc.tile_pool(name="pr", bufs=6) as prp, \
                 tc.tile_pool(name="rcp", bufs=HPG * n_j) as rcp, \
                 tc.tile_pool(name="sm", bufs=3) as smp, \
                 tc.tile_pool(name="psS", bufs=3, space="PSUM") as psS, \
                 tc.tile_pool(name="psB", bufs=2, space="PSUM") as psB, \
                 tc.tile_pool(name="psB1", bufs=1, space="PSUM") as psB1:
                rcs = {}
                for h in range(HPG):
                    if h == 0:
                        qh, kh, vh = q0_sb, k0_sb, v0_sb
                    else:
                        qh = hqp.tile([P, s], F32R, tag="qh")
                        nc.sync.dma_start(qh[:], qh_d[h][:])
                        kh = hqp.tile([P, s], F32R, tag="kh")
                        nc.sync.dma_start(kh[:], kh_d[h][:])
                        vh = hqp.tile([P, s // P, HD], F32R, tag="vh")
                        nc.scalar.dma_start(vh[:], vh_d[h][:])
                    for jq in range(n_j):
                        live = [i for i in range(n_i) if classes[(jq, i)] != 'skip']
                        qsl = slice(jq * SQ, (jq + 1) * SQ)
                        at_ps = psB.tile([P, SQ], F32, tag="at")
                        dn_ps = psB.tile([1, SQ], F32, tag="dn")
                        for n, i in enumerate(live):
                            sc = psS.tile([P, SQ], F32, tag="sc")
                            nc.tensor.matmul(
                                sc[:], kh[:, i * P:(i + 1) * P], qh[:, qsl],
                                start=True, stop=True)
                            pr = prp.tile([P, SQ], F32R, tag="pr")
                            nc.scalar.activation(
                                pr[:], sc[:], mybir.ActivationFunctionType.Exp,
                                scale=SCALE)
                            cls = classes[(jq, i)]
                            if isinstance(cls, int):
                                nc.vector.tensor_tensor(
                                    pr[:], pr[:], mblk_t[:, cls, :], MULT)
                            nc.tensor.matmul(
                                at_ps[:], vh[:, i, :], pr[:],
                                start=(n == 0), stop=(n == len(live) - 1),
                                skip_group_check=True)
                            nc.tensor.matmul(
                                dn_ps[:], ones_r[:], pr[:],
                                start=(n == 0), stop=(n == len(live) - 1),
                                skip_group_check=True)
                        # stash the unnormalized row block + its denominators;
                        # normalization is deferred so the PE never waits on it
                        nc.scalar.activation(
                            attn_t[:, h, qsl], at_ps[:],
                            mybir.ActivationFunctionType.Copy)
                        rc = rcp.tile([1, SQ], F32R, tag="rc")
                        with nc.allow_low_precision(reason="f32r is bitwise f32"):
                            nc.vector.reciprocal(rc[:], dn_ps[:])
                        rcs[(h, jq)] = rc
                for (h, jq), rc in rcs.items():
                    qsl = slice(jq * SQ, (jq + 1) * SQ)
                    bc_ps = psB1.tile([P, SQ], F32, tag="bc")
                    nc.tensor.matmul(bc_ps[:], ones_f[:], rc[:],
                                     start=True, stop=True)
                    bc_sb = smp.tile([P, SQ], F32, tag="bcs")
                    nc.scalar.activation(
                        bc_sb[:], bc_ps[:], mybir.ActivationFunctionType.Copy)
                    nc.vector.tensor_tensor(
                        attn_t[:, h, qsl], attn_t[:, h, qsl], bc_sb[:], MULT)

              # ---------------- Phase C: output projection ----------------
              with tc.tile_pool(name="og", bufs=2) as ogp, \
                   tc.tile_pool(name="psC", bufs=4, space="PSUM") as psC:
                  for st in range(n_i):
                      og = ogp.tile([P, d], F32, tag="og")
                      for dch in range(d // SQ):
                          po = psC.tile([P, SQ], F32, tag="po")
                          for ct in range(HPG):
                              nc.tensor.matmul(
                                  po[:], attn_t[:, ct, st * P:(st + 1) * P],
                                  wo_t[:, ct, dch * SQ:(dch + 1) * SQ],
                                  start=(ct == 0), stop=(ct == HPG - 1))
                          nc.scalar.activation(
                              og[:, dch * SQ:(dch + 1) * SQ], po[:],
                              mybir.ActivationFunctionType.Copy)
                      nc.sync.dma_start(out[st * P:(st + 1) * P, :], og[:])

    nc.compile()
    return nc


def _prep_host(inputs):
    """Shard + transpose the full inputs into 8 per-core input maps."""
    x = np.asarray(inputs["x"], np.float32)
    wq = np.asarray(inputs["wq"], np.float32)
    wk = np.asarray(inputs["wk"], np.float32)
    wv = np.asarray(inputs["wv"], np.float32)
    wo = np.asarray(inputs["wo"], np.float32)
    cos = np.asarray(inputs["cos"], np.float32)
    sin = np.asarray(inputs["sin"], np.float32)
    mask = np.asarray(inputs["mask"], np.float32)
    start_p = int(inputs["start_p"])

    s = x.shape[1]
    cos_u = cos[start_p:start_p + s]          # (s, HD/2)
    sin_u = sin[start_p:start_p + s]

    # rotate-half channel permutation within each head: [evens, odds]
    perm = np.concatenate(
        [h * HD + np.concatenate([np.arange(0, HD, 2), np.arange(1, HD, 2)])
         for h in range(H)])

    cosP = np.ascontiguousarray(
        np.concatenate([cos_u.T, cos_u.T], axis=0))          # (128, s)
    sinSw = np.ascontiguousarray(
        np.concatenate([sin_u.T, -sin_u.T], axis=0))         # (128, s)

    classes, pats = _classify_mask(mask)
    onesd = np.ones((P, 1), np.float32)

    in_maps = []
    for b in range(B):
        xTp = _pre_x(np.ascontiguousarray(x[b].T))
        for g in range(GROUPS):
            rows = perm[g * C:(g + 1) * C]
            in_maps.append({
                "xT": xTp,
                "wqT": _pre_w(wq[rows, :].T),
                "wkT": _pre_w(wk[rows, :].T),
                "wvT": _pre_w(wv[g * C:(g + 1) * C, :].T),
                "woT": _pre_w(wo[:, g * C:(g + 1) * C].T),
                "cosP": cosP,
                "sinSw": sinSw,
                "mblk": np.ascontiguousarray(pats.transpose(1, 0, 2)),
                "onesd": onesd,
            })
    return in_maps, classes, pats


def _run(inputs, trace=False):
    in_maps, classes, pats = _prep_host(inputs)
    key = (pats.shape[0], tuple(sorted(classes.items())))
    if key not in _PROGRAM_CACHE:
        _PROGRAM_CACHE[key] = _build(classes, pats.shape[0])
    nc = _PROGRAM_CACHE[key]
    res = bass_utils.run_bass_kernel_spmd(
        nc, in_maps, core_ids=list(range(NCORES)), trace=trace)
    out = np.zeros((B, S, D), np.float32)
    for b in range(B):
        acc = res.results[b * GROUPS]["out"].astype(np.float32).copy()
        for g in range(1, GROUPS):
            acc += res.results[b * GROUPS + g]["out"]
        out[b] = acc
    return out, res


def kernel(**inputs):
    out, _ = _run(inputs, trace=False)
    return out


# revision 14
# speedup vs baseline: 1.3028x; 1.0113x over previous
"""Trainium2 Bass kernel for nn_Attention_51307679318359.

Multi-head attention (B=2, S=2048, D=2048, H=16, HD=128) with RoPE and an
additive mask, sharded over 8 NeuronCores as (batch x head-group): each core
computes 1 batch and 4 heads (512 channels), producing a partial output that
the host sums over head-groups.

Per-core dataflow (all activations kept transposed, channels on partitions):
  QT = wqT' @ xT, KT = wkT' @ xT  (rotate-half permuted weights), RoPE applied
  on the PSUM output via DVE; V = xT' @ wvT; spilled to DRAM as f32r (per-head
  tiles so the attention phase can start as soon as head 0 is written).
  Per head: scoresT(sk,sq) = KT_h-slice.T @ QT_h (fp32r, one matmul per
  128x512 block), exp on ACT, multiplicative exp(mask) patterns on the mixed
  blocks, AV + ones-vector denominator accumulated in PSUM, normalization via
  reciprocal + K=1 f32r broadcast matmul + DVE multiply. Out-proj contracts
  the 4 head tiles against woT.
Fully-masked score blocks (causal upper triangle) are skipped based on a
host-side classification of the mask into skip / plain / pattern blocks.
"""

import math

import numpy as np

import concourse.bass as bass
import concourse.mybir as mybir
import concourse.tile as tile
from concourse import bacc
from concourse import bass_utils

F32 = mybir.dt.float32
F32R = mybir.dt.float32r
ADD = mybir.AluOpType.add
MULT = mybir.AluOpType.mult

B, S, D = 2, 2048, 2048
H, HD = 16, 128
NCORES = 8
GROUPS = NCORES // B          # 4 head-groups
HPG = H // GROUPS             # 4 heads per group
C = HPG * HD                  # 512 per-core channels
P = 128
CH_A = 256                    # phase-A s-chunk width
SQ = 512                      # phase-B sq-chunk width
SCALE = 1.0 / math.sqrt(HD)
NEG_THRESH = -1e8             # "masked out" threshold

_PROGRAM_CACHE = {}


def _pre_w(wT):
    """(d, c) row-major -> (128, d//128, c) partition-major contiguous."""
    d, c = wT.shape
    return np.ascontiguousarray(wT.reshape(d // P, P, c).transpose(1, 0, 2))


def _pre_x(xT):
    """(d, s) -> (s//CH_A, 128, d//128, CH_A) chunk-major contiguous."""
    d, s = xT.shape
    return np.ascontiguousarray(
        xT.reshape(d // P, P, s // CH_A, CH_A).transpose(2, 1, 0, 3))


def _classify_mask(mask):
    """Classify transposed-mask blocks (sk-tile i x sq-chunk j) and dedupe the
    mixed patterns. Returns (classes, patterns): classes[(j, i)] is
    'skip' | 'plain' | pattern index; patterns is (nblk, 128, SQ) f32 holding
    exp(maskT block) (multiplicative masking applied to the probs after exp).
    """
    maskT = np.ascontiguousarray(mask.T)
    n_j = mask.shape[0] // SQ
    n_i = mask.shape[0] // P
    classes = {}
    patterns = []
    pat_idx = {}

    def add_pattern(blk):
        key = blk.tobytes()
        if key not in pat_idx:
            pat_idx[key] = len(patterns)
            with np.errstate(over='ignore'):
                patterns.append(np.exp(blk.astype(np.float64)).astype(np.float32))
        return pat_idx[key]

    for j in range(n_j):
        for i in range(n_i):
            blk = maskT[i * P:(i + 1) * P, j * SQ:(j + 1) * SQ]
            if np.all(blk == 0.0):
                classes[(j, i)] = 'plain'
            elif np.all(blk <= NEG_THRESH):
                classes[(j, i)] = 'skip'
            else:
                classes[(j, i)] = add_pattern(blk)
    # every sq position must keep at least one live sk tile, else softmax
    # denominators vanish; fall back to no skipping in that degenerate case
    if any(all(classes[(j, i)] == 'skip' for i in range(n_i)) for j in range(n_j)):
        for j in range(n_j):
            for i in range(n_i):
                if classes[(j, i)] == 'skip':
                    blk = maskT[i * P:(i + 1) * P, j * SQ:(j + 1) * SQ]
                    classes[(j, i)] = add_pattern(blk)
    pats = np.stack(patterns, 0).astype(np.float32) if patterns else \
        np.zeros((1, P, SQ), np.float32)
    return classes, pats


def _build(classes, nblk, s=S, d=D):
    """Build + compile the per-core SPMD program."""
    nkt = d // P
    n_j = s // SQ
    n_i = s // P
    n_ja = s // CH_A

    nc = bacc.Bacc("TRN2", target_bir_lowering=False, debug=False)
    xT = nc.dram_tensor("xT", (n_ja, P, nkt, CH_A), F32, kind="ExternalInput")
    wqT = nc.dram_tensor("wqT", (P, nkt, C), F32, kind="ExternalInput")
    wkT = nc.dram_tensor("wkT", (P, nkt, C), F32, kind="ExternalInput")
    wvT = nc.dram_tensor("wvT", (P, nkt, C), F32, kind="ExternalInput")
    woT = nc.dram_tensor("woT", (P, HPG, d), F32, kind="ExternalInput")
    cosP = nc.dram_tensor("cosP", (HD, s), F32, kind="ExternalInput")
    sinSw = nc.dram_tensor("sinSw", (HD, s), F32, kind="ExternalInput")
    mblk = nc.dram_tensor("mblk", (P, nblk, SQ), F32, kind="ExternalInput")
    onesd = nc.dram_tensor("onesd", (P, 1), F32, kind="ExternalInput")
    out = nc.dram_tensor("out", (s, d), F32, kind="ExternalOutput")

    with tile.TileContext(nc) as tc:
        with tc.tile_pool(name="dram", bufs=1, space="DRAM") as dram, \
             tc.tile_pool(name="const", bufs=1) as const:
            qh_d = [dram.tile([P, s], F32R, name=f"qh_d{h}") for h in range(HPG)]
            kh_d = [dram.tile([P, s], F32R, name=f"kh_d{h}") for h in range(HPG)]
            vh_d = [dram.tile([P, s // P, HD], F32R, name=f"vh_d{h}")
                    for h in range(HPG)]

            ones_r = const.tile([P, 1], F32R)
            nc.gpsimd.dma_start(ones_r[:], onesd[:].bitcast(F32R))
            ones_f = const.tile([1, P], F32R)
            nc.gpsimd.dma_start(ones_f[:],
                                onesd[:].rearrange("a b -> b a").bitcast(F32R))
            mblk_t = const.tile([P, nblk, SQ], F32R)
            nc.gpsimd.dma_start(mblk_t[:], mblk[:].bitcast(F32R))
            q0_sb = const.tile([P, s], F32R)
            k0_sb = const.tile([P, s], F32R)
            v0_sb = const.tile([P, s // P, HD], F32R)

            # ---------------- Phase A: QKV projections + RoPE ----------------
            with tc.tile_pool(name="wres", bufs=1) as wres, \
                 tc.tile_pool(name="xc", bufs=2) as xcp, \
                 tc.tile_pool(name="trig", bufs=1) as trig, \
                 tc.tile_pool(name="ptmp", bufs=2) as ptmp, \
                 tc.tile_pool(name="stg", bufs=3) as stg, \
                 tc.tile_pool(name="psA", bufs=2, space="PSUM") as psA:
                wq_t = wres.tile([P, nkt, C], F32R)
                nc.sync.dma_start(wq_t[:], wqT[:].bitcast(F32R))
                cos_t = trig.tile([P, s], F32)
                sin_t = trig.tile([P, s], F32)
                wk_t = wres.tile([P, nkt, C], F32R)
                wv_t = wres.tile([P, nkt, C], F32R)

                for j in range(n_ja):
                    sl = slice(j * CH_A, (j + 1) * CH_A)
                    xc = xcp.tile([P, nkt, CH_A], F32R, tag="xc")
                    nc.scalar.dma_start(xc[:], xT[j].bitcast(F32R))
                    for (wt, dst, sb0) in ((wq_t, qh_d, q0_sb), (wk_t, kh_d, k0_sb)):
                        for ct in range(HPG):
                            ps = psA.tile([P, CH_A], F32, tag="ps_qk")
                            for k in range(nkt):
                                nc.tensor.matmul(
                                    ps[:], wt[:, k, ct * P:(ct + 1) * P],
                                    xc[:, k, :],
                                    start=(k == 0), stop=(k == nkt - 1))
                            if j == 0 and wt is wq_t and ct == 0:
                                # deferred so the PE start event only gates on
                                # wq + the first x chunk (emitted after the
                                # first matmul group, before anything reads them)
                                nc.sync.dma_start(wk_t[:], wkT[:].bitcast(F32R))
                                nc.scalar.dma_start(cos_t[:], cosP[:])
                                nc.scalar.dma_start(sin_t[:], sinSw[:])
                            if j == 0 and wt is wq_t and ct == 1:
                                nc.sync.dma_start(wv_t[:], wvT[:].bitcast(F32R))
                            # RoPE (rotate-half layout):
                            #   out_top = x0*cos - x1*sin ; out_bot = x1*cos + x0*sin
                            t1 = ptmp.tile([P, CH_A], F32, tag="t1")
                            nc.vector.tensor_tensor(t1[:], ps[:], cos_t[:, sl], MULT)
                            t2 = ptmp.tile([P, CH_A], F32, tag="t2")
                            nc.vector.tensor_tensor(
                                t2[0:64, :], ps[64:128, :], sin_t[64:128, sl], MULT)
                            nc.vector.tensor_tensor(
                                t2[64:128, :], ps[0:64, :], sin_t[0:64, sl], MULT)
                            if ct == 0:
                                # head 0 stays SBUF-resident across the A->B boundary
                                nc.vector.tensor_tensor(
                                    sb0[:, sl], t1[:], t2[:], ADD)
                            else:
                                ro = stg.tile([P, CH_A], F32R, tag="ro")
                                nc.vector.tensor_tensor(ro[:], t1[:], t2[:], ADD)
                                nc.sync.dma_start(dst[ct][:, sl], ro[:])
                    for st2 in range(CH_A // P):
                        st = (j * CH_A) // P + st2
                        psv = psA.tile([P, C], F32, tag="ps_v")
                        for k in range(nkt):
                            nc.tensor.matmul(
                                psv[:], xc[:, k, st2 * P:(st2 + 1) * P],
                                wv_t[:, k, :],
                                start=(k == 0), stop=(k == nkt - 1))
                        nc.vector.tensor_copy(v0_sb[:, st, :], psv[:, 0:HD])
                        vo = stg.tile([P, C - HD], F32R, tag="vo")
                        nc.vector.tensor_copy(vo[:], psv[:, HD:])
                        for h in range(1, HPG):
                            nc.sync.dma_start(
                                vh_d[h][:, st, :], vo[:, (h - 1) * HD:h * HD])

            # ---------------- Phase B: attention per head ----------------
            with tc.tile_pool(name="attn", bufs=1) as attnp:
              attn_t = attnp.tile([P, HPG, s], F32R)
              wo_t = attnp.tile([P, HPG, d], F32R)
              nc.sync.dma_start(wo_t[:], woT[:].bitcast(F32R))
              with tc.tile_pool(name="hq", bufs=2) as hqp, \
                 tc.tile_pool(name="pr", bufs=6) as prp, \
                 tc.tile_pool(name="rcp", bufs=HPG * n_j) as rcp, \
                 tc.tile_pool(name="sm", bufs=3) as smp, \
                 tc.tile_pool(name="psS", bufs=3, space="PSUM") as psS, \
                 tc.tile_pool(name="psB", bufs=2, space="PSUM") as psB, \
                 tc.tile_pool(name="psB1", bufs=1, space="PSUM") as psB1:
                rcs = {}
                for h in range(HPG):
                    if h == 0:
                        qh, kh, vh = q0_sb, k0_sb, v0_sb
                    else:
                        qh = hqp.tile([P, s], F32R, tag="qh")
                        nc.sync.dma_start(qh[:], qh_d[h][:])
                        kh = hqp.tile([P, s], F32R, tag="kh")
                        nc.sync.dma_start(kh[:], kh_d[h][:])
                        vh = hqp.tile([P, s // P, HD], F32R, tag="vh")
                        nc.scalar.dma_start(vh[:], vh_d[h][:])
                    for jq in range(n_j):
                        live = [i for i in range(n_i) if classes[(jq, i)] != 'skip']
                        qsl = slice(jq * SQ, (jq + 1) * SQ)
                        at_ps = psB.tile([P, SQ], F32, tag="at")
                        dn_ps = psB.tile([1, SQ], F32, tag="dn")
                        for n, i in enumerate(live):
                            sc = psS.tile([P, SQ], F32, tag="sc")
                            nc.tensor.matmul(
                                sc[:], kh[:, i * P:(i + 1) * P], qh[:, qsl],
                                start=True, stop=True)
                            pr = prp.tile([P, SQ], F32R, tag="pr")
                            nc.scalar.activation(
                                pr[:], sc[:], mybir.ActivationFunctionType.Exp,
                                scale=SCALE)
                            cls = classes[(jq, i)]
                            if isinstance(cls, int):
                                nc.vector.tensor_tensor(
                                    pr[:], pr[:], mblk_t[:, cls, :], MULT)
                            nc.tensor.matmul(
                                at_ps[:], vh[:, i, :], pr[:],
                                start=(n == 0), stop=(n == len(live) - 1),
                                skip_group_check=True)
                            nc.tensor.matmul(
                                dn_ps[:], ones_r[:], pr[:],
                                start=(n == 0), stop=(n == len(live) - 1),
                                skip_group_check=True)
                        # stash the unnormalized row block + its denominators;
                        # normalization is deferred so the PE never waits on it
                        nc.scalar.activation(
                            attn_t[:, h, qsl], at_ps[:],
                            mybir.ActivationFunctionType.Copy)
                        rc = rcp.tile([1, SQ], F32R, tag="rc")
                        with nc.allow_low_precision(reason="f32r is bitwise f32"):
                            nc.vector.reciprocal(rc[:], dn_ps[:])
                        rcs[(h, jq)] = rc
                for (h, jq), rc in rcs.items():
                    qsl = slice(jq * SQ, (jq + 1) * SQ)
                    bc_ps = psB1.tile([P, SQ], F32, tag="bc")
                    nc.tensor.matmul(bc_ps[:], ones_f[:], rc[:],
                                     start=True, stop=True)
                    bc_sb = smp.tile([P, SQ], F32, tag="bcs")
                    nc.scalar.activation(
                        bc_sb[:], bc_ps[:], mybir.ActivationFunctionType.Copy)
                    nc.vector.tensor_tensor(
                        attn_t[:, h, qsl], attn_t[:, h, qsl], bc_sb[:], MULT)

              # ---------------- Phase C: output projection ----------------
              with tc.tile_pool(name="og", bufs=2) as ogp, \
                   tc.tile_pool(name="psC", bufs=4, space="PSUM") as psC:
                  for st in range(n_i):
                      og = ogp.tile([P, d], F32, tag="og")
                      for dch in range(d // SQ):
                          po = psC.tile([P, SQ], F32, tag="po")
                          for ct in range(HPG):
                              nc.tensor.matmul(
                                  po[:], attn_t[:, ct, st * P:(st + 1) * P],
                                  wo_t[:, ct, dch * SQ:(dch + 1) * SQ],
                                  start=(ct == 0), stop=(ct == HPG - 1))
                          nc.scalar.activation(
                              og[:, dch * SQ:(dch + 1) * SQ], po[:],
                              mybir.ActivationFunctionType.Copy)
                      nc.sync.dma_start(out[st * P:(st + 1) * P, :], og[:])

    nc.compile()
    return nc


def _prep_host(inputs):
    """Shard + transpose the full inputs into 8 per-core input maps."""
    x = np.asarray(inputs["x"], np.float32)
    wq = np.asarray(inputs["wq"], np.float32)
    wk = np.asarray(inputs["wk"], np.float32)
    wv = np.asarray(inputs["wv"], np.float32)
    wo = np.asarray(inputs["wo"], np.float32)
    cos = np.asarray(inputs["cos"], np.float32)
    sin = np.asarray(inputs["sin"], np.float32)
    mask = np.asarray(inputs["mask"], np.float32)
    start_p = int(inputs["start_p"])

    s = x.shape[1]
    cos_u = cos[start_p:start_p + s]          # (s, HD/2)
    sin_u = sin[start_p:start_p + s]

    # rotate-half channel permutation within each head: [evens, odds]
    perm = np.concatenate(
        [h * HD + np.concatenate([np.arange(0, HD, 2), np.arange(1, HD, 2)])
         for h in range(H)])

    cosP = np.ascontiguousarray(
        np.concatenate([cos_u.T, cos_u.T], axis=0))          # (128, s)
    sinSw = np.ascontiguousarray(
        np.concatenate([sin_u.T, -sin_u.T], axis=0))         # (128, s)

    classes, pats = _classify_mask(mask)
    onesd = np.ones((P, 1), np.float32)

    in_maps = []
    for b in range(B):
        xTp = _pre_x(np.ascontiguousarray(x[b].T))
        for g in range(GROUPS):
            rows = perm[g * C:(g + 1) * C]
            in_maps.append({
                "xT": xTp,
                "wqT": _pre_w(wq[rows, :].T),
                "wkT": _pre_w(wk[rows, :].T),
                "wvT": _pre_w(wv[g * C:(g + 1) * C, :].T),
                "woT": _pre_w(wo[:, g * C:(g + 1) * C].T),
                "cosP": cosP,
                "sinSw": sinSw,
                "mblk": np.ascontiguousarray(pats.transpose(1, 0, 2)),
                "onesd": onesd,
            })
    return in_maps, classes, pats


def _run(inputs, trace=False):
    in_maps, classes, pats = _prep_host(inputs)
    key = (pats.shape[0], tuple(sorted(classes.items())))
    if key not in _PROGRAM_CACHE:
        _PROGRAM_CACHE[key] = _build(classes, pats.shape[0])
    nc = _PROGRAM_CACHE[key]
    res = bass_utils.run_bass_kernel_spmd(
        nc, in_maps, core_ids=list(range(NCORES)), trace=trace)
    out = np.zeros((B, S, D), np.float32)
    for b in range(B):
        acc = res.results[b * GROUPS]["out"].astype(np.float32).copy()
        for g in range(1, GROUPS):
            acc += res.results[b * GROUPS + g]["out"]
        out[b] = acc
    return out, res


def kernel(**inputs):
    out, _ = _run(inputs, trace=False)
    return out
